# revision 9
# baseline (speedup 1.0000x reference)
"""Trainium2 Bass kernel for nn_MemeGeneratorM2 (img-conditioned char-LSTM).

Model: tok embeddings + img embedding -> linear proj -> LSTM(H=1024, T=512,
teacher-forced) -> vocab logits (V=128) -> argmax preds.

Strategy (8 NeuronCores, data-parallel over batch, 16 rows/core):
  * All pre-LSTM linear algebra is folded on the host into two gather tables:
      gates_x[b,t] = Ttok[tok[b,t]] + Timg[img[b]]   (biases folded into Timg)
    so the device only gathers rows and runs the recurrence + output head.
  * Recurrent matmul per step: out[b, 4H] += h[b, :] @ W_hh.T, computed as
    lhsT = h^T chunks [128, 16] (stationary), rhs = W_hh^T resident in SBUF
    [128p, 8k, 4096] (moving), accumulated over 8 K-tiles into 8 PSUM banks.
    float32r dtype: full-rate streaming with ~13-bit mantissa (measured).
  * PSUM gates [16, 4096] are evacuated+gx-added by DVE into SBUF, DMA-
    relayouted to a [128, 512] layout (partition = h-chunk*16 + batch) for
    full-lane elementwise LSTM cell math, then h is re-transposed to h^T with
    one full-tile PE transpose.
  * h^T history is ring-buffered and flushed to HBM; a post-pass computes
    logits = hs @ W_fc.T (+bias) in out[V, token] orientation, transposes
    tiles back for an argmax (max + max_index) over the free axis.

W_hh^T / gate-table columns are host-permuted to block order
(n = half*4 + gate), so PSUM bank n holds gate `n%4` for h-half `n//4` and
each relayout DMA j needs only the 4 banks of its half.
"""
import sys
import numpy as np
from contextlib import ExitStack

for _p in ("/opt/trn_rl_repo", "/root/.axon_site/_ro/trn_rl_repo"):
    if _p not in sys.path:
        import os
        if os.path.isdir(_p):
            sys.path.insert(0, _p)

import concourse.bass as bass
import concourse.tile as tile
from concourse import mybir
from concourse.bass_utils import run_bass_kernel_spmd
from concourse.masks import make_identity

dt = mybir.dt
AF = mybir.ActivationFunctionType
OP = mybir.AluOpType

B, T, V, NIMG = 128, 512, 128, 1000
E, IMG_E, H = 128, 32, 1024
G4 = 4 * H                      # 4096 gate width
NCORES = 8
BL = B // NCORES                # 16 batch rows per core
KT = H // 128                   # 8 contraction tiles
FL = 8                          # h-history flush window (steps)
PD = 2                          # gates_x gather prefetch depth (steps)

_WAIT_CAPS = {}
_WAIT_CAP_DEFAULT = 1


def _patch_drain():
    """walrus in this container has small per-instruction sync-wait budgets;
    the Tile kernel-tail Drain waits on every active proc. Split its waits
    onto single-wait NoOp carriers emitted just before it."""
    if getattr(tile.TileContext, "_drain_patched", False):
        return
    from concourse.vector_clock import ScopedClock

    def _patched(self, tick_clock, wait_clock):
        nops = [self.nc.sync.nop(nofuse=True, hint="drain_split") for _ in range(28)]
        drain_inst = self.nc.sync.drain()
        wait_clock.add_sem_waits(
            drain_inst.ins, ScopedClock({None: tick_clock.global_clock})
        )
        si = drain_inst.ins.sync_info
        w = list(si.on_wait or [])
        if len(w) > 1:
            rest, keep = w[:-1], w[-1:]
            for i, nop in enumerate(nops):
                if i >= len(rest):
                    break
                nop.ins.sync_info = mybir.SyncInfo(on_wait=[rest[i]], on_update=[])
            si.on_wait = keep
            drain_inst.ins.sync_info = si
        self.nc.all_engine_barrier()
        popped = self.nc._tile_sem_poison_stack.pop()
        assert popped is self._sem_poison
        self.nc.clear_and_free_semaphores(list(self.sems.allocated().values()))
        self.nc.all_engine_barrier()

    tile.TileContext._drain_and_barrier = _patched
    tile.TileContext._drain_patched = True


def _split_fat_waits(nc):
    """Post-pass: any instruction whose sync-wait list exceeds the walrus
    per-instruction budget gets excess waits moved to same-engine NoOps
    inserted directly before it (sequencers execute waits in order)."""
    n_split = 0
    for f in nc.m.functions:
        for blk in f.blocks:
            insts = blk.instructions
            out = []
            changed = False
            for ins in insts:
                si = ins.sync_info
                w = list(si.on_wait or []) if si is not None else []
                cap = _WAIT_CAPS.get(type(ins).__name__, _WAIT_CAP_DEFAULT)
                if len(w) > cap:
                    rest, keep = w[: len(w) - cap], w[len(w) - cap:]
                    for wi in rest:
                        n_split += 1
                        nop = mybir.InstNoOp(
                            name=f"bass-waitsplit-{n_split}", ins=[], outs=[]
                        )
                        nop.engine = ins.engine
                        nop.sync_info = mybir.SyncInfo(on_wait=[wi], on_update=[])
                        out.append(nop)
                    si.on_wait = keep
                    ins.sync_info = si
                    changed = True
                out.append(ins)
            if changed:
                insts.clear()
                insts.extend(out)
    return n_split


def build_program(t_steps=T, phase=2, feats=31):
    _patch_drain()
    nc = bass.Bass("TRN2", target_bir_lowering=False, debug=False,
                   num_devices=NCORES)
    f32, f32r, i32 = dt.float32, dt.float32r, dt.int32

    tok_d = nc.dram_tensor("tok", [BL, T], i32, kind="ExternalInput")
    img_d = nc.dram_tensor("imgix", [BL, 1], i32, kind="ExternalInput")
    Ttok_d = nc.dram_tensor("Ttok", [V, G4], f32, kind="ExternalInput")
    Timg_d = nc.dram_tensor("Timg", [NIMG, G4], f32, kind="ExternalInput")
    Whh_d = nc.dram_tensor("WhhT", [KT, 128, G4], f32r, kind="ExternalInput")
    Wfc_d = nc.dram_tensor("WfcT", [KT, 128, V], f32r, kind="ExternalInput")
    bfc_d = nc.dram_tensor("bfc", [V, 1], f32, kind="ExternalInput")
    logits_d = nc.dram_tensor("logits_vt", [V, BL * t_steps], f32,
                              kind="ExternalOutput")
    preds_d = nc.dram_tensor("preds", [BL * t_steps], i32, kind="ExternalOutput")
    hsT_d = nc.dram_tensor("hsT", [KT, 128, BL * t_steps], f32r)

    with tile.TileContext(nc) as tc, ExitStack() as ctx:
        const = ctx.enter_context(tc.tile_pool(name="const", bufs=1))
        gxp = ctx.enter_context(tc.tile_pool(name="gx", bufs=PD))
        work = ctx.enter_context(tc.tile_pool(name="work", bufs=1))
        lrhs = ctx.enter_context(tc.tile_pool(name="lrhs", bufs=3))
        lout = ctx.enter_context(tc.tile_pool(name="lout", bufs=2))
        ps_mm = ctx.enter_context(tc.tile_pool(name="psmm", bufs=3, space="PSUM"))
        ps_tp = ctx.enter_context(tc.tile_pool(name="pstp", bufs=2, space="PSUM"))

        # ---- resident state / constants ----
        Whh_sb = const.tile([128, KT, G4], f32r)
        nc.sync.dma_start(Whh_sb[:], Whh_d[:].rearrange("k p c -> p k c"))
        ident = const.tile([128, 128], f32)
        make_identity(nc, ident[:])
        tok_sb = const.tile([BL, T], i32)
        nc.sync.dma_start(tok_sb[:], tok_d[:])
        img_sb = const.tile([BL, 1], i32)
        nc.sync.dma_start(img_sb[:], img_d[:])
        R_sb = const.tile([BL, G4], f32)
        nc.gpsimd.indirect_dma_start(
            out=R_sb[:], out_offset=None, in_=Timg_d[:],
            in_offset=bass.IndirectOffsetOnAxis(ap=img_sb[:, 0:1], axis=0),
        )
        Wfc_sb = const.tile([128, KT, V], f32r)
        nc.sync.dma_start(Wfc_sb[:], Wfc_d[:].rearrange("k p v -> p k v"))
        bfc_sb = const.tile([V, 1], f32)
        nc.sync.dma_start(bfc_sb[:], bfc_d[:])

        gall = const.tile([128, 512], f32)    # relayouted raw gates
        acts = const.tile([128, 512], f32)    # activated gates (i,f,g,o)
        c_sb = const.tile([128, 128], f32)
        t1 = const.tile([128, 128], f32)
        t2 = const.tile([128, 128], f32)
        tct = const.tile([128, 128], f32)
        h_sb = const.tile([128, 128], f32)
        hT_sb = const.tile([128, 128], f32r)
        ring = const.tile([128, FL, 128], f32r)
        nc.vector.memzero(c_sb[:])

        # ---- gates_x gather prefetch ----
        gx_tiles = {}

        def prefetch(tt):
            if tt >= t_steps:
                return
            g = gxp.tile([BL, G4], dt.float32, tag="gx")
            nc.gpsimd.indirect_dma_start(
                out=g[:], out_offset=None, in_=Ttok_d[:],
                in_offset=bass.IndirectOffsetOnAxis(ap=tok_sb[:, tt:tt + 1], axis=0),
            )
            nc.vector.tensor_add(g[:], g[:], R_sb[:])
            gx_tiles[tt] = g

        for tt in range(PD):
            prefetch(tt)

        # ---- the scan ----
        for t in range(t_steps if phase >= 1 else 0):
            bt = gx_tiles.pop(t)
            if t > 0 and (feats & 1):
                for n in range(8):
                    psg = ps_mm.tile([BL, 512], dt.float32, tag="mm")
                    for k in range(KT):
                        nc.tensor.matmul(
                            psg[:],
                            hT_sb[:, 16 * k:16 * (k + 1)],
                            Whh_sb[:, k, 512 * n:512 * (n + 1)],
                            start=(k == 0), stop=(k == KT - 1),
                        )
                    nc.vector.tensor_tensor(
                        bt[:, 512 * n:512 * (n + 1)],
                        bt[:, 512 * n:512 * (n + 1)],
                        psg[:], OP.add,
                    )
            prefetch(t + PD)
            # relayout [16, (half gate jloc hl)] -> [(j b), (gate hl)]
            gsv = bt[:].rearrange("b (h g j l) -> b h g j l", h=2, g=4, j=4)
            for j in range((feats >> 1) & 8 or 0 if not (feats & 2) else 8):
                nc.scalar.dma_start(
                    gall[16 * j:16 * (j + 1), :].rearrange("b (g l) -> b g l", g=4),
                    gsv[:, j >> 2, :, j & 3, :],
                )
            if not (feats & 4):
                continue
            nc.scalar.activation(acts[:, 0:256], gall[:, 0:256], AF.Sigmoid)
            nc.scalar.activation(acts[:, 256:384], gall[:, 256:384], AF.Tanh)
            nc.scalar.activation(acts[:, 384:512], gall[:, 384:512], AF.Sigmoid)
            nc.vector.tensor_mul(t1[:], acts[:, 128:256], c_sb[:])
            nc.vector.tensor_mul(t2[:], acts[:, 0:128], acts[:, 256:384])
            nc.vector.tensor_add(c_sb[:], t1[:], t2[:])
            nc.scalar.activation(tct[:], c_sb[:], AF.Tanh)
            nc.vector.tensor_mul(h_sb[:], acts[:, 384:512], tct[:])
            # h -> h^T: 4 PE transposes of [32,128] slabs into one PSUM bank
            if not (feats & 8):
                continue
            pst = ps_tp.tile([128, 128], dt.float32, tag="tp")
            nc.tensor.transpose(pst[:], h_sb[:], ident[:])
            nc.vector.tensor_copy(hT_sb[:], pst[:])
            if not (feats & 16):
                continue
            nc.vector.tensor_copy(ring[:, t % FL, :], hT_sb[:])
            if t % FL == FL - 1:
                a0 = (t - FL + 1) * BL
                for k in range(KT):
                    nc.sync.dma_start(
                        hsT_d[k, :, a0:a0 + FL * BL].rearrange(
                            "p (w b) -> p w b", b=BL),
                        ring[:, :, 16 * k:16 * (k + 1)],
                    )

        # ---- logits + argmax ----
        n_tok = BL * t_steps
        if phase < 2:
            n_tok = 0
        preds_all = const.tile([128, max(n_tok, 512) // 128], f32)
        for ci in range(n_tok // 512):
            rk = []
            for k in range(KT):
                r = lrhs.tile([128, 512], f32r, tag="lrhs")
                nc.sync.dma_start(r[:], hsT_d[k, :, 512 * ci:512 * (ci + 1)])
                rk.append(r)
            psl = ps_mm.tile([128, 512], dt.float32, tag="mm")
            for k in range(KT):
                nc.tensor.matmul(psl[:], Wfc_sb[:, k, :], rk[k][:],
                                 start=(k == 0), stop=(k == KT - 1))
            sbl = lout.tile([128, 512], f32, tag="sbl")
            nc.scalar.activation(sbl[:], psl[:], AF.Identity, bias=bfc_sb[:, 0:1])
            nc.sync.dma_start(logits_d[:, 512 * ci:512 * (ci + 1)], sbl[:])
            for s in range(4):
                pstt = ps_tp.tile([128, 128], dt.float32, tag="tp")
                nc.tensor.transpose(pstt[:], sbl[:, 128 * s:128 * (s + 1)], ident[:])
                sbt = lout.tile([128, 128], f32, tag="sbt")
                nc.vector.tensor_copy(sbt[:], pstt[:])
                mx8 = lout.tile([128, 8], f32, tag="mx8")
                ix8 = lout.tile([128, 8], dt.uint32, tag="ix8")
                nc.vector.max(mx8[:], sbt[:])
                nc.vector.max_index(ix8[:], mx8[:], sbt[:])
                nc.vector.tensor_copy(preds_all[:, 4 * ci + s:4 * ci + s + 1],
                                      ix8[:, 0:1])
        if n_tok:
            psp = ps_tp.tile([128, 128], dt.float32, tag="tp")
            nc.tensor.transpose(psp[:n_tok // 128, :], preds_all[:], ident[:])
            preds_i = const.tile([n_tok // 128, 128], i32)
            nc.vector.tensor_copy(preds_i[:], psp[:n_tok // 128, :])
            nc.sync.dma_start(preds_d[:].rearrange("(a b) -> a b", b=128), preds_i[:])

    return nc


_CACHE = {}
LAST_RESULTS = None


def _get_program(t_steps=T):
    if t_steps not in _CACHE:
        nc = build_program(t_steps)
        _split_fat_waits(nc)
        _CACHE[t_steps] = nc
    return _CACHE[t_steps]


def _fold_tables(emb_img, emb_char, W_proj, b_proj, W_ih, W_hh, b_ih, b_hh,
                 W_fc):
    """Fold embedding + input projection + input-to-gate matmul into gather
    tables, and permute gate columns to device block order (half, gate)."""
    Kimg = W_ih @ W_proj[:, :IMG_E]            # [4H, 32]
    Ktok = W_ih @ W_proj[:, IMG_E:]            # [4H, E]
    bias = W_ih @ b_proj + b_ih + b_hh         # [4H]
    Ttok = emb_char @ Ktok.T                   # [V, 4H]
    Timg = emb_img @ Kimg.T + bias             # [NIMG, 4H]
    # column permutation: new block n = half*4 + gate  <-  gate*1024 + half*512
    perm = np.concatenate([
        np.arange(g * H + hf * (H // 2), g * H + hf * (H // 2) + H // 2)
        for hf in range(2) for g in range(4)
    ])
    WhhT = np.ascontiguousarray(W_hh.T[:, perm]).reshape(KT, 128, G4)
    return (np.ascontiguousarray(Ttok[:, perm]),
            np.ascontiguousarray(Timg[:, perm]),
            WhhT,
            np.ascontiguousarray(W_fc.T).reshape(KT, 128, V))


def kernel(input_img, x, label, emb_img, emb_char, W_proj, b_proj,
           W_ih, W_hh, b_ih, b_hh, W_fc, b_fc, _t_steps=T):
    input_img = np.asarray(input_img)
    x = np.asarray(x); label = np.asarray(label)
    f32 = np.float32
    Ttok, Timg, WhhT, WfcT = _fold_tables(
        np.asarray(emb_img, f32), np.asarray(emb_char, f32),
        np.asarray(W_proj, f32), np.asarray(b_proj, f32),
        np.asarray(W_ih, f32), np.asarray(W_hh, f32),
        np.asarray(b_ih, f32), np.asarray(b_hh, f32), np.asarray(W_fc, f32))
    bfc = np.asarray(b_fc, f32).reshape(V, 1)
    tok = np.concatenate([x[:, :1], label[:, :-1]], axis=1).astype(np.int32)
    imgix = input_img.astype(np.int32).reshape(B, 1)

    nc = _get_program(_t_steps)
    in_maps = []
    for c in range(NCORES):
        sl = slice(c * BL, (c + 1) * BL)
        in_maps.append({
            "tok": np.ascontiguousarray(tok[sl]),
            "imgix": np.ascontiguousarray(imgix[sl]),
            "Ttok": Ttok, "Timg": Timg, "WhhT": WhhT, "WfcT": WfcT,
            "bfc": bfc,
        })
    import os as _os
    _trace = bool(int(_os.environ.get("BASS_LSTM_TRACE", "0")))
    res = run_bass_kernel_spmd(nc, in_maps, list(range(NCORES)), trace=_trace)
    global LAST_RESULTS
    LAST_RESULTS = res
    ts = _t_steps
    logits = np.empty((B, ts, V), np.float32)
    preds = np.empty((B, ts), np.int32)
    for c in range(NCORES):
        lv = res.results[c]["logits_vt"]          # [V, BL*ts], token = t*BL+b
        pv = res.results[c]["preds"]              # [BL*ts]
        logits[c * BL:(c + 1) * BL] = lv.reshape(V, ts, BL).transpose(2, 1, 0)
        preds[c * BL:(c + 1) * BL] = pv.reshape(ts, BL).T
    idx_dtype = np.asarray(x).dtype
    return logits, preds.astype(idx_dtype)


# revision 10
# speedup vs baseline: 48.2138x; 48.2138x over previous
"""Trainium2 Bass kernel for nn_MemeGeneratorM2 (img-conditioned char-LSTM).

Model: tok embeddings + img embedding -> linear proj -> LSTM(H=1024, T=512,
teacher-forced) -> vocab logits (V=128) -> argmax preds.

Strategy (8 NeuronCores, data-parallel over batch, 16 rows/core):
  * All pre-LSTM linear algebra is folded on the host into two gather tables:
      gates_x[b,t] = Ttok[tok[b,t]] + Timg[img[b]]   (biases folded into Timg)
    so the device only gathers rows and runs the recurrence + output head.
  * Recurrent matmul per step: out[b, 4H] += h[b, :] @ W_hh.T, computed as
    lhsT = h^T chunks [128, 16] (stationary), rhs = W_hh^T resident in SBUF
    [128p, 8k, 4096] (moving), accumulated over 8 K-tiles into 8 PSUM banks.
    float32r dtype: full-rate streaming with ~13-bit mantissa (measured).
  * PSUM gates [16, 4096] are evacuated+gx-added by DVE into SBUF, DMA-
    relayouted to a [128, 512] layout (partition = h-chunk*16 + batch) for
    full-lane elementwise LSTM cell math, then h is re-transposed to h^T with
    one full-tile PE transpose.
  * h^T history is ring-buffered and flushed to HBM; a post-pass computes
    logits = hs @ W_fc.T (+bias) in out[V, token] orientation, transposes
    tiles back for an argmax (max + max_index) over the free axis.

W_hh^T / gate-table columns are host-permuted to block order
(n = half*4 + gate), so PSUM bank n holds gate `n%4` for h-half `n//4` and
each relayout DMA j needs only the 4 banks of its half.
"""
import sys
import numpy as np
from contextlib import ExitStack

for _p in ("/opt/trn_rl_repo", "/root/.axon_site/_ro/trn_rl_repo"):
    if _p not in sys.path:
        import os
        if os.path.isdir(_p):
            sys.path.insert(0, _p)

import concourse.bass as bass
import concourse.tile as tile
from concourse import mybir
from concourse.bass_utils import run_bass_kernel_spmd
from concourse.masks import make_identity

dt = mybir.dt
AF = mybir.ActivationFunctionType
OP = mybir.AluOpType

B, T, V, NIMG = 128, 512, 128, 1000
E, IMG_E, H = 128, 32, 1024
G4 = 4 * H                      # 4096 gate width
NCORES = 8
BL = B // NCORES                # 16 batch rows per core
KT = H // 128                   # 8 contraction tiles
FL = 8                          # h-history flush window (steps)
PD = 2                          # gates_x gather prefetch depth (steps)

_WAIT_CAPS = {}
_WAIT_CAP_DEFAULT = 1


def _patch_drain():
    """walrus in this container has small per-instruction sync-wait budgets;
    the Tile kernel-tail Drain waits on every active proc. Split its waits
    onto single-wait NoOp carriers emitted just before it."""
    if getattr(tile.TileContext, "_drain_patched", False):
        return
    from concourse.vector_clock import ScopedClock

    def _patched(self, tick_clock, wait_clock):
        nops = [self.nc.sync.nop(nofuse=True, hint="drain_split") for _ in range(28)]
        drain_inst = self.nc.sync.drain()
        wait_clock.add_sem_waits(
            drain_inst.ins, ScopedClock({None: tick_clock.global_clock})
        )
        si = drain_inst.ins.sync_info
        w = list(si.on_wait or [])
        if len(w) > 1:
            rest, keep = w[:-1], w[-1:]
            for i, nop in enumerate(nops):
                if i >= len(rest):
                    break
                nop.ins.sync_info = mybir.SyncInfo(on_wait=[rest[i]], on_update=[])
            si.on_wait = keep
            drain_inst.ins.sync_info = si
        self.nc.all_engine_barrier()
        popped = self.nc._tile_sem_poison_stack.pop()
        assert popped is self._sem_poison
        self.nc.clear_and_free_semaphores(list(self.sems.allocated().values()))
        self.nc.all_engine_barrier()

    tile.TileContext._drain_and_barrier = _patched
    tile.TileContext._drain_patched = True


def _split_fat_waits(nc):
    """Post-pass: any instruction whose sync-wait list exceeds the walrus
    per-instruction budget gets excess waits moved to same-engine NoOps
    inserted directly before it (sequencers execute waits in order)."""
    n_split = 0
    for f in nc.m.functions:
        for blk in f.blocks:
            insts = blk.instructions
            out = []
            changed = False
            for ins in insts:
                si = ins.sync_info
                w = list(si.on_wait or []) if si is not None else []
                cap = _WAIT_CAPS.get(type(ins).__name__, _WAIT_CAP_DEFAULT)
                if len(w) > cap:
                    rest, keep = w[: len(w) - cap], w[len(w) - cap:]
                    for wi in rest:
                        n_split += 1
                        nop = mybir.InstNoOp(
                            name=f"bass-waitsplit-{n_split}", ins=[], outs=[]
                        )
                        nop.engine = ins.engine
                        nop.sync_info = mybir.SyncInfo(on_wait=[wi], on_update=[])
                        out.append(nop)
                    si.on_wait = keep
                    ins.sync_info = si
                    changed = True
                out.append(ins)
            if changed:
                insts.clear()
                insts.extend(out)
    return n_split


def build_program(t_steps=T, phase=2, feats=31):
    _patch_drain()
    nc = bass.Bass("TRN2", target_bir_lowering=False, debug=False,
                   num_devices=NCORES)
    f32, f32r, i32 = dt.float32, dt.float32r, dt.int32

    tok_d = nc.dram_tensor("tok", [BL, T], i32, kind="ExternalInput")
    img_d = nc.dram_tensor("imgix", [BL, 1], i32, kind="ExternalInput")
    Ttok_d = nc.dram_tensor("Ttok", [V, G4], f32, kind="ExternalInput")
    Timg_d = nc.dram_tensor("Timg", [NIMG, G4], f32, kind="ExternalInput")
    Whh_d = nc.dram_tensor("WhhT", [KT, 128, G4], f32r, kind="ExternalInput")
    Wfc_d = nc.dram_tensor("WfcT", [KT, 128, V], f32r, kind="ExternalInput")
    bfc_d = nc.dram_tensor("bfc", [V, 1], f32, kind="ExternalInput")
    logits_d = nc.dram_tensor("logits_vt", [V, BL * t_steps], f32,
                              kind="ExternalOutput")
    preds_d = nc.dram_tensor("preds", [BL * t_steps], i32, kind="ExternalOutput")
    hsT_d = nc.dram_tensor("hsT", [KT, 128, BL * t_steps], f32r)

    with tile.TileContext(nc) as tc, ExitStack() as ctx:
        const = ctx.enter_context(tc.tile_pool(name="const", bufs=1))
        gxp = ctx.enter_context(tc.tile_pool(name="gx", bufs=PD))
        work = ctx.enter_context(tc.tile_pool(name="work", bufs=1))
        lrhs = ctx.enter_context(tc.tile_pool(name="lrhs", bufs=3))
        lout = ctx.enter_context(tc.tile_pool(name="lout", bufs=2))
        ps_mm = ctx.enter_context(tc.tile_pool(name="psmm", bufs=3, space="PSUM"))
        ps_tp = ctx.enter_context(tc.tile_pool(name="pstp", bufs=2, space="PSUM"))

        # ---- resident state / constants ----
        Whh_sb = const.tile([128, KT, G4], f32r)
        nc.sync.dma_start(Whh_sb[:], Whh_d[:].rearrange("k p c -> p k c"))
        ident = const.tile([128, 128], f32)
        make_identity(nc, ident[:])
        tok_sb = const.tile([BL, T], i32)
        nc.sync.dma_start(tok_sb[:], tok_d[:])
        img_sb = const.tile([BL, 1], i32)
        nc.sync.dma_start(img_sb[:], img_d[:])
        R_sb = const.tile([BL, G4], f32)
        nc.gpsimd.indirect_dma_start(
            out=R_sb[:], out_offset=None, in_=Timg_d[:],
            in_offset=bass.IndirectOffsetOnAxis(ap=img_sb[:, 0:1], axis=0),
        )
        Wfc_sb = const.tile([128, KT, V], f32r)
        nc.sync.dma_start(Wfc_sb[:], Wfc_d[:].rearrange("k p v -> p k v"))
        bfc_sb = const.tile([V, 1], f32)
        nc.sync.dma_start(bfc_sb[:], bfc_d[:])

        gall = const.tile([128, 512], f32)    # relayouted raw gates
        acts = const.tile([128, 512], f32)    # activated gates (i,f,g,o)
        c_sb = const.tile([128, 128], f32)
        t1 = const.tile([128, 128], f32)
        t2 = const.tile([128, 128], f32)
        tct = const.tile([128, 128], f32)
        h_sb = const.tile([128, 128], f32)
        hT_sb = const.tile([128, 128], f32r)
        ring = const.tile([128, FL, 128], f32r)
        nc.vector.memzero(c_sb[:])

        # ---- gates_x gather prefetch ----
        gx_tiles = {}

        def prefetch(tt):
            if tt >= t_steps:
                return
            g = gxp.tile([BL, G4], dt.float32, tag="gx")
            nc.gpsimd.indirect_dma_start(
                out=g[:], out_offset=None, in_=Ttok_d[:],
                in_offset=bass.IndirectOffsetOnAxis(ap=tok_sb[:, tt:tt + 1], axis=0),
            )
            nc.vector.tensor_add(g[:], g[:], R_sb[:])
            gx_tiles[tt] = g

        for tt in range(PD):
            prefetch(tt)

        # ---- the scan ----
        for t in range(t_steps if phase >= 1 else 0):
            bt = gx_tiles.pop(t)
            if t > 0 and (feats & 1):
                for n in range(8):
                    psg = ps_mm.tile([BL, 512], dt.float32, tag="mm")
                    for k in range(KT):
                        nc.tensor.matmul(
                            psg[:],
                            hT_sb[:, 16 * k:16 * (k + 1)],
                            Whh_sb[:, k, 512 * n:512 * (n + 1)],
                            start=(k == 0), stop=(k == KT - 1),
                        )
                    nc.vector.tensor_tensor(
                        bt[:, 512 * n:512 * (n + 1)],
                        bt[:, 512 * n:512 * (n + 1)],
                        psg[:], OP.add,
                    )
            prefetch(t + PD)
            # relayout [16, (half gate jloc hl)] -> [(j b), (gate hl)]
            gsv = bt[:].rearrange("b (h g j l) -> b h g j l", h=2, g=4, j=4)
            for j in range((feats >> 1) & 8 or 0 if not (feats & 2) else 8):
                nc.scalar.dma_start(
                    gall[16 * j:16 * (j + 1), :].rearrange("b (g l) -> b g l", g=4),
                    gsv[:, j >> 2, :, j & 3, :],
                )
            if not (feats & 4):
                continue
            nc.scalar.activation(acts[:, 0:256], gall[:, 0:256], AF.Sigmoid)
            nc.scalar.activation(acts[:, 256:384], gall[:, 256:384], AF.Tanh)
            nc.scalar.activation(acts[:, 384:512], gall[:, 384:512], AF.Sigmoid)
            nc.vector.tensor_mul(t1[:], acts[:, 128:256], c_sb[:])
            nc.vector.tensor_mul(t2[:], acts[:, 0:128], acts[:, 256:384])
            nc.vector.tensor_add(c_sb[:], t1[:], t2[:])
            nc.scalar.activation(tct[:], c_sb[:], AF.Tanh)
            nc.vector.tensor_mul(h_sb[:], acts[:, 384:512], tct[:])
            # h -> h^T: 4 PE transposes of [32,128] slabs into one PSUM bank
            if not (feats & 8):
                continue
            pst = ps_tp.tile([128, 128], dt.float32, tag="tp")
            nc.tensor.transpose(pst[:], h_sb[:], ident[:])
            nc.vector.tensor_copy(hT_sb[:], pst[:])
            if not (feats & 16):
                continue
            nc.vector.tensor_copy(ring[:, t % FL, :], hT_sb[:])
            if t % FL == FL - 1:
                a0 = (t - FL + 1) * BL
                for k in range(KT):
                    nc.sync.dma_start(
                        hsT_d[k, :, a0:a0 + FL * BL].rearrange(
                            "p (w b) -> p w b", b=BL),
                        ring[:, :, 16 * k:16 * (k + 1)],
                    )

        # ---- logits + argmax ----
        n_tok = BL * t_steps
        if phase < 2:
            n_tok = 0
        preds_all = const.tile([128, max(n_tok, 512) // 128], f32)
        for ci in range(n_tok // 512):
            rk = []
            for k in range(KT):
                r = lrhs.tile([128, 512], f32r, tag="lrhs")
                nc.sync.dma_start(r[:], hsT_d[k, :, 512 * ci:512 * (ci + 1)])
                rk.append(r)
            psl = ps_mm.tile([128, 512], dt.float32, tag="mm")
            for k in range(KT):
                nc.tensor.matmul(psl[:], Wfc_sb[:, k, :], rk[k][:],
                                 start=(k == 0), stop=(k == KT - 1))
            sbl = lout.tile([128, 512], f32, tag="sbl")
            nc.scalar.activation(sbl[:], psl[:], AF.Identity, bias=bfc_sb[:, 0:1])
            nc.sync.dma_start(logits_d[:, 512 * ci:512 * (ci + 1)], sbl[:])
            for s in range(4):
                pstt = ps_tp.tile([128, 128], dt.float32, tag="tp")
                nc.tensor.transpose(pstt[:], sbl[:, 128 * s:128 * (s + 1)], ident[:])
                sbt = lout.tile([128, 128], f32, tag="sbt")
                nc.vector.tensor_copy(sbt[:], pstt[:])
                mx8 = lout.tile([128, 8], f32, tag="mx8")
                ix8 = lout.tile([128, 8], dt.uint32, tag="ix8")
                nc.vector.max(mx8[:], sbt[:])
                nc.vector.max_index(ix8[:], mx8[:], sbt[:])
                nc.vector.tensor_copy(preds_all[:, 4 * ci + s:4 * ci + s + 1],
                                      ix8[:, 0:1])
        if n_tok:
            psp = ps_tp.tile([128, 128], dt.float32, tag="tp")
            nc.tensor.transpose(psp[:n_tok // 128, :], preds_all[:], ident[:])
            preds_i = const.tile([n_tok // 128, 128], i32)
            nc.vector.tensor_copy(preds_i[:], psp[:n_tok // 128, :])
            nc.sync.dma_start(preds_d[:].rearrange("(a b) -> a b", b=128), preds_i[:])

    return nc


_CACHE = {}
LAST_RESULTS = None


def _get_program(t_steps=T):
    if t_steps not in _CACHE:
        nc = build_program(t_steps)
        _split_fat_waits(nc)
        _CACHE[t_steps] = nc
    return _CACHE[t_steps]


def _fold_tables(emb_img, emb_char, W_proj, b_proj, W_ih, W_hh, b_ih, b_hh,
                 W_fc):
    """Fold embedding + input projection + input-to-gate matmul into gather
    tables, and permute gate columns to device block order (half, gate)."""
    Kimg = W_ih @ W_proj[:, :IMG_E]            # [4H, 32]
    Ktok = W_ih @ W_proj[:, IMG_E:]            # [4H, E]
    bias = W_ih @ b_proj + b_ih + b_hh         # [4H]
    Ttok = emb_char @ Ktok.T                   # [V, 4H]
    Timg = emb_img @ Kimg.T + bias             # [NIMG, 4H]
    # column permutation: new block n = half*4 + gate  <-  gate*1024 + half*512
    perm = np.concatenate([
        np.arange(g * H + hf * (H // 2), g * H + hf * (H // 2) + H // 2)
        for hf in range(2) for g in range(4)
    ])
    WhhT = np.ascontiguousarray(W_hh.T[:, perm]).reshape(KT, 128, G4)
    return (np.ascontiguousarray(Ttok[:, perm]),
            np.ascontiguousarray(Timg[:, perm]),
            WhhT,
            np.ascontiguousarray(W_fc.T).reshape(KT, 128, V))


def kernel(input_img, x, label, emb_img, emb_char, W_proj, b_proj,
           W_ih, W_hh, b_ih, b_hh, W_fc, b_fc, _t_steps=T):
    input_img = np.asarray(input_img)
    x = np.asarray(x); label = np.asarray(label)
    f32 = np.float32
    Ttok, Timg, WhhT, WfcT = _fold_tables(
        np.asarray(emb_img, f32), np.asarray(emb_char, f32),
        np.asarray(W_proj, f32), np.asarray(b_proj, f32),
        np.asarray(W_ih, f32), np.asarray(W_hh, f32),
        np.asarray(b_ih, f32), np.asarray(b_hh, f32), np.asarray(W_fc, f32))
    bfc = np.asarray(b_fc, f32).reshape(V, 1)
    tok = np.concatenate([x[:, :1], label[:, :-1]], axis=1).astype(np.int32)
    imgix = input_img.astype(np.int32).reshape(B, 1)

    nc = _get_program(_t_steps)
    in_maps = []
    for c in range(NCORES):
        sl = slice(c * BL, (c + 1) * BL)
        in_maps.append({
            "tok": np.ascontiguousarray(tok[sl]),
            "imgix": np.ascontiguousarray(imgix[sl]),
            "Ttok": Ttok, "Timg": Timg, "WhhT": WhhT, "WfcT": WfcT,
            "bfc": bfc,
        })
    import os as _os
    _trace = bool(int(_os.environ.get("BASS_LSTM_TRACE", "0")))
    res = run_bass_kernel_spmd(nc, in_maps, list(range(NCORES)), trace=_trace)
    global LAST_RESULTS
    LAST_RESULTS = res
    ts = _t_steps
    logits = np.empty((B, ts, V), np.float32)
    preds = np.empty((B, ts), np.int32)
    for c in range(NCORES):
        lv = res.results[c]["logits_vt"]          # [V, BL*ts], token = t*BL+b
        pv = res.results[c]["preds"]              # [BL*ts]
        logits[c * BL:(c + 1) * BL] = lv.reshape(V, ts, BL).transpose(2, 1, 0)
        preds[c * BL:(c + 1) * BL] = pv.reshape(ts, BL).T
    idx_dtype = np.asarray(x).dtype
    return logits, preds.astype(idx_dtype)


def bench_exec_ns(iters=6, t_steps=T, **inputs):
    """Time the on-device execution (warm, inputs pre-staged) in ns.

    Mirrors bass2jax.run_bass_via_pjrt's multi-core path but keeps the jitted
    executable and device-resident inputs alive so repeated calls time only
    dispatch + NEFF execution."""
    import time as _time
    import jax
    from jax.sharding import Mesh, PartitionSpec
    from jax.experimental.shard_map import shard_map
    from concourse import bass2jax
    from concourse.bass2jax import _bass_exec_p, install_neuronx_cc_hook, \
        partition_id_tensor

    nc = _get_program(t_steps)
    in_maps = _make_in_maps(**inputs)
    install_neuronx_cc_hook()
    n_cores = NCORES
    partition_name = nc.partition_id_tensor.name if nc.partition_id_tensor else None
    in_names, out_names, out_avals, zero_outs = [], [], [], []
    for alloc in nc.m.functions[0].allocations:
        if not isinstance(alloc, mybir.MemoryLocationSet):
            continue
        name = alloc.memorylocations[0].name
        if alloc.kind == "ExternalInput":
            if name != partition_name:
                in_names.append(name)
        elif alloc.kind == "ExternalOutput":
            out_names.append(name)
            shape = tuple(alloc.tensor_shape)
            np_dt = mybir.dt.np(alloc.dtype)
            out_avals.append(jax.core.ShapedArray(shape, np_dt))
            zero_outs.append(np.zeros(shape, np_dt))
    n_params = len(in_names)
    n_outs = len(out_avals)
    all_in_names = list(in_names) + list(out_names)
    if partition_name is not None:
        all_in_names.append(partition_name)
    donate = tuple(range(n_params, n_params + n_outs))

    def _body(*args):
        operands = list(args)
        if partition_name is not None:
            operands.append(partition_id_tensor())
        outs = _bass_exec_p.bind(
            *operands, out_avals=tuple(out_avals), in_names=tuple(all_in_names),
            out_names=tuple(out_names), lowering_input_output_aliases=(),
            sim_require_finite=True, sim_require_nnan=True, nc=nc)
        return tuple(outs)

    devices = jax.devices()[:n_cores]
    mesh = Mesh(np.asarray(devices), ("core",))
    sharded = jax.jit(
        shard_map(_body, mesh=mesh, in_specs=(PartitionSpec("core"),) * (n_params + n_outs),
                  out_specs=(PartitionSpec("core"),) * n_outs, check_rep=False),
        donate_argnums=donate, keep_unused=True)
    concat_in = [np.concatenate([np.asarray(in_maps[c][nm]) for c in range(n_cores)], axis=0)
                 for nm in in_names]
    sh_in = jax.sharding.NamedSharding(mesh, PartitionSpec("core"))
    dev_in = [jax.device_put(a, sh_in) for a in concat_in]

    def one_call():
        zs = [jax.device_put(np.zeros((n_cores * z.shape[0], *z.shape[1:]), z.dtype), sh_in)
              for z in zero_outs]
        for z in zs:
            z.block_until_ready()
        t0 = _time.perf_counter()
        outs = sharded(*dev_in, *zs)
        for o in outs:
            o.block_until_ready()
        return _time.perf_counter() - t0, outs

    one_call()  # warm-up/compile
    times = []
    outs = None
    for _ in range(iters):
        dtm, outs = one_call()
        times.append(dtm)
    return int(min(times) * 1e9), times, outs, out_names


def _make_in_maps(input_img, x, label, emb_img, emb_char, W_proj, b_proj,
                  W_ih, W_hh, b_ih, b_hh, W_fc, b_fc):
    f32 = np.float32
    Ttok, Timg, WhhT, WfcT = _fold_tables(
        np.asarray(emb_img, f32), np.asarray(emb_char, f32),
        np.asarray(W_proj, f32), np.asarray(b_proj, f32),
        np.asarray(W_ih, f32), np.asarray(W_hh, f32),
        np.asarray(b_ih, f32), np.asarray(b_hh, f32), np.asarray(W_fc, f32))
    bfc = np.asarray(b_fc, f32).reshape(V, 1)
    x = np.asarray(x); label = np.asarray(label)
    tok = np.concatenate([x[:, :1], label[:, :-1]], axis=1).astype(np.int32)
    imgix = np.asarray(input_img).astype(np.int32).reshape(B, 1)
    in_maps = []
    for c in range(NCORES):
        sl = slice(c * BL, (c + 1) * BL)
        in_maps.append({
            "tok": np.ascontiguousarray(tok[sl]),
            "imgix": np.ascontiguousarray(imgix[sl]),
            "Ttok": Ttok, "Timg": Timg, "WhhT": WhhT, "WfcT": WfcT,
            "bfc": bfc,
        })
    return in_maps
